# revision 8
# baseline (speedup 1.0000x reference)
"""HalfKP-NNUE embedding-bag + MLP kernel for 8 Trainium2 NeuronCores.

Strategy (pure data-parallel over the batch, B=8192 -> 1024 rows/core):
  The embedding gather+sum over K=30 indices into a 640-row table is
  re-expressed as a dense matmul with a multi-hot "counts" matrix:
      sum0[b, :] = sum_k w1[idx[b,k], :]  ==  counts[b, :] @ w1
  counts[b, c] = multiplicity of c in idx[b, :].

  Per core / per table:
    1. DMA idx [1024, 30] int32 -> SBUF tiles [128, 8, 30].
    2. VectorE: occurrence numbers pre[b,k] = #{k' <= k : idx[b,k']==idx[b,k]}
       via an all-pairs sliding-window equality + binary-tree add reduction.
    3. GpSimd local_scatter per 128-row tile: counts[b, idx[b,k]] = pre[b,k]
       (duplicate slots resolved by last-write-wins -> multiplicity).
    4. TensorE: transpose counts (fp16) into PSUM, evacuate as fp32 countsT.
    5. TensorE: S.T[e, b] = sum_c w1[c, e] * countsT[c, b] (fp32 matmul,
       accumulated over 5 c-chunks of 128), fused ReLU on PSUM evacuation.
    6. Tiny MLP (512->32->32->1) as fp32 matmuls contracted on partitions.
  Outputs are exact-fp32-level accurate (counts are exact small ints).
"""

import numpy as np

HIDDEN = 256
TABLE = 640
B = 8192
K = 30
NCORES = 8
BLOC = B // NCORES          # 1024 rows per core
NTILES = BLOC // 128        # 8 tiles of 128 rows
CCHUNKS = TABLE // 128      # 5 contraction chunks
MLPH = 32

_COMPILED = {}


def _build_bass():
    import concourse.bass as bass
    import concourse.mybir as mybir
    import concourse.tile as tile
    from concourse import library_config
    from contextlib import ExitStack

    dt = mybir.dt
    AF = mybir.ActivationFunctionType
    OP = mybir.AluOpType

    nc = bass.Bass()

    idx0_d = nc.declare_dram_parameter("idx0", [BLOC, K], dt.int32, isOutput=False)
    idx1_d = nc.declare_dram_parameter("idx1", [BLOC, K], dt.int32, isOutput=False)
    w1_d = nc.declare_dram_parameter("w1", [2, TABLE, HIDDEN], dt.float32, isOutput=False)
    fc2wT_d = nc.declare_dram_parameter("fc2wT", [2 * HIDDEN, MLPH], dt.float32, isOutput=False)
    fc3wT_d = nc.declare_dram_parameter("fc3wT", [MLPH, MLPH], dt.float32, isOutput=False)
    fc4wT_d = nc.declare_dram_parameter("fc4wT", [MLPH, 1], dt.float32, isOutput=False)
    fc2b_d = nc.declare_dram_parameter("fc2b", [MLPH, 1], dt.float32, isOutput=False)
    fc3b_d = nc.declare_dram_parameter("fc3b", [MLPH, 1], dt.float32, isOutput=False)
    fc4b_d = nc.declare_dram_parameter("fc4b", [1, 1], dt.float32, isOutput=False)
    out_d = nc.declare_dram_parameter("out", [1, BLOC], dt.float32, isOutput=True)

    with tile.TileContext(nc) as tc, ExitStack() as ctx:
        const_pool = ctx.enter_context(tc.tile_pool(name="const", bufs=1))
        work_pool = ctx.enter_context(tc.tile_pool(name="work", bufs=2))
        ct_pool = ctx.enter_context(tc.tile_pool(name="ct", bufs=1))
        h_pool = ctx.enter_context(tc.tile_pool(name="h", bufs=1))
        psum_ct = ctx.enter_context(tc.tile_pool(name="psum_ct", bufs=2, space="PSUM"))
        psum_st = ctx.enter_context(tc.tile_pool(name="psum_st", bufs=2, space="PSUM"))
        psum_mlp = ctx.enter_context(tc.tile_pool(name="psum_mlp", bufs=2, space="PSUM"))

        # GPSIMD ucode library holding the local_scatter kernel must be
        # resident before any scatter executes (Pool engine program order).
        nc.gpsimd.load_library(library_config.local_scatter)

        # ---- constants / weights ----
        w1sb = const_pool.tile([128, 2, CCHUNKS, HIDDEN], dt.float32)
        nc.sync.dma_start(
            out=w1sb[:],
            in_=w1_d[:].rearrange("s (cc p) e -> p s cc e", p=128),
        )
        fc2wT = const_pool.tile([128, 4, MLPH], dt.float32)
        nc.sync.dma_start(
            out=fc2wT[:], in_=fc2wT_d[:].rearrange("(dc p) u -> p dc u", p=128)
        )
        fc3wT = const_pool.tile([MLPH, MLPH], dt.float32)
        nc.sync.dma_start(out=fc3wT[:], in_=fc3wT_d[:])
        fc4wT = const_pool.tile([MLPH, 1], dt.float32)
        nc.sync.dma_start(out=fc4wT[:], in_=fc4wT_d[:])
        fc2b = const_pool.tile([MLPH, 1], dt.float32)
        nc.sync.dma_start(out=fc2b[:], in_=fc2b_d[:])
        fc3b = const_pool.tile([MLPH, 1], dt.float32)
        nc.sync.dma_start(out=fc3b[:], in_=fc3b_d[:])
        fc4b = const_pool.tile([1, 1], dt.float32)
        nc.sync.dma_start(out=fc4b[:], in_=fc4b_d[:])

        ident_d = nc.inline_tensor(np.eye(128, dtype=np.float16), name="ident")
        ident = const_pool.tile([128, 128], dt.float16)
        nc.sync.dma_start(out=ident[:], in_=ident_d[:])

        # h layout: [128, dc, BLOC] where dc = 2*table + e_chunk
        hsb = h_pool.tile([128, 4, BLOC], dt.float32)

        for t, idx_d in enumerate((idx0_d, idx1_d)):
            idx32 = work_pool.tile([128, NTILES, K], dt.int32, tag="idx32")
            nc.sync.dma_start(
                out=idx32[:], in_=idx_d[:].rearrange("(ti p) k -> p ti k", p=128)
            )
            # int16 copy for scatter addressing; fp16 copy for exact compares
            idx16 = work_pool.tile([128, NTILES, K], dt.int16, tag="idx16")
            nc.vector.tensor_copy(idx16[:], idx32[:])
            idxf = work_pool.tile([128, NTILES, K], dt.float16, tag="idxf")
            nc.vector.tensor_copy(idxf[:], idx32[:])

            # padded buffer: [0:30]=-1 sentinel, [30:60]=idx, [60:64]=-1
            pad = work_pool.tile([128, NTILES, 64], dt.float16, tag="pad")
            nc.vector.memset(pad[:], -1.0)
            nc.vector.tensor_copy(pad[:, :, K : 2 * K], idxf[:])

            # all-pairs equality with sliding window:
            # eq[p, ti, k, j] = (idx[p,ti,k] == pad[p, ti, k+1+j]), j=0..29
            # window covers idx[k-29..k] (j=29 is the self-match).
            eq = work_pool.tile([128, NTILES, K, 32], dt.float16, tag="eq")
            nc.vector.memset(eq[:, :, :, 30:32], 0.0)
            eq_ap = eq[:, :, :, 0:K]
            in0 = idxf[:].unsqueeze(3).broadcast_to([128, NTILES, K, K])
            pad_full = pad[:]
            win = type(pad_full)(
                tensor=pad_full.tensor,
                offset=1,
                ap=[list(pad_full.ap[0]), [64, NTILES], [1, K], [1, K]],
            )
            nc.vector.tensor_tensor(eq_ap, in0, win, OP.is_equal)

            # binary-tree reduce along j: 32 -> 16 -> 8 -> 4 -> 2 -> 1
            w = 32
            while w > 1:
                h = w // 2
                nc.vector.tensor_tensor(
                    eq[:, :, :, 0:h], eq[:, :, :, 0:h], eq[:, :, :, h:w], OP.add
                )
                w = h
            # pre[p, ti, k] = occurrence number of idx[p,ti,k] (1..30)
            pre = work_pool.tile([128, NTILES, K], dt.float16, tag="pre")
            nc.vector.tensor_copy(pre[:], eq[:, :, :, 0:1].squeeze(3))

            counts = work_pool.tile([128, NTILES, TABLE], dt.float16, tag="counts")
            for ti in range(NTILES):
                nc.gpsimd.local_scatter(
                    counts[:, ti, :],
                    pre[:, ti, :],
                    idx16[:, ti, :],
                    channels=128,
                    num_elems=TABLE,
                    num_idxs=K,
                )

            # transpose counts tile-block-wise into PSUM (fp16 pass-through)
            ctsb = ct_pool.tile([128, 2, CCHUNKS, BLOC], dt.float32, tag="ctsb")
            for cc in range(CCHUNKS):
                ctp = psum_ct.tile([128, BLOC], dt.float16, tag="ctp")
                for ti in range(NTILES):
                    nc.tensor.transpose(
                        ctp[:, ti * 128 : (ti + 1) * 128],
                        counts[:, ti, cc * 128 : (cc + 1) * 128],
                        ident[:],
                    )
                nc.any.tensor_copy(ctsb[:, t, cc, :], ctp[:])

            # S.T[e, b] = sum_c w1[c, e] * countsT[c, b], fp32
            for ec in range(2):
                for hh in range(2):
                    st = psum_st.tile([128, 512], dt.float32, tag="st")
                    for cc in range(CCHUNKS):
                        nc.tensor.matmul(
                            st[:],
                            w1sb[:, t, cc, ec * 128 : (ec + 1) * 128],
                            ctsb[:, t, cc, hh * 512 : (hh + 1) * 512],
                            start=(cc == 0),
                            stop=(cc == CCHUNKS - 1),
                        )
                    nc.scalar.activation(
                        hsb[:, 2 * t + ec, hh * 512 : (hh + 1) * 512],
                        st[:],
                        AF.Relu,
                    )

        # ---- MLP ----
        h2sb = h_pool.tile([MLPH, BLOC], dt.float32)
        for hh in range(2):
            p2 = psum_mlp.tile([MLPH, 512], dt.float32, tag="mlp")
            for dc in range(4):
                nc.tensor.matmul(
                    p2[:],
                    fc2wT[:, dc, :],
                    hsb[:, dc, hh * 512 : (hh + 1) * 512],
                    start=(dc == 0),
                    stop=(dc == 3),
                )
            nc.scalar.activation(
                h2sb[:, hh * 512 : (hh + 1) * 512], p2[:], AF.Relu, bias=fc2b[:]
            )
        h3sb = h_pool.tile([MLPH, BLOC], dt.float32)
        for hh in range(2):
            p3 = psum_mlp.tile([MLPH, 512], dt.float32, tag="mlp")
            nc.tensor.matmul(
                p3[:], fc3wT[:], h2sb[:, hh * 512 : (hh + 1) * 512], start=True, stop=True
            )
            nc.scalar.activation(
                h3sb[:, hh * 512 : (hh + 1) * 512], p3[:], AF.Relu, bias=fc3b[:]
            )
        osb = h_pool.tile([1, BLOC], dt.float32)
        for hh in range(2):
            p4 = psum_mlp.tile([1, 512], dt.float32, tag="mlp")
            nc.tensor.matmul(
                p4[:], fc4wT[:], h3sb[:, hh * 512 : (hh + 1) * 512], start=True, stop=True
            )
            nc.scalar.activation(
                osb[:, hh * 512 : (hh + 1) * 512], p4[:], AF.Identity, bias=fc4b[:]
            )
        nc.sync.dma_start(out=out_d[:], in_=osb[:])

    # Populate .instr bytes for extended-inst InstISA subclasses
    # (LocalScatter); without this walrus fails with "ISA wrong length".
    mybir.codegen_inst_isa_subclasses(nc)
    # TRN2: instructions carry a limited number of sem-wait slots; spill
    # excess matmul waits to ldweights and split the rest via event sems.
    import bass_rust
    bass_rust.move_matmul_waits_to_ldweights(nc.m)
    bass_rust.generate_event_semaphores(nc)
    return nc


def _prep_in_maps(inputs):
    idx0 = np.ascontiguousarray(np.asarray(inputs["idx0_batch"]).astype(np.int32))
    idx1 = np.ascontiguousarray(np.asarray(inputs["idx1_batch"]).astype(np.int32))
    w1 = np.ascontiguousarray(np.asarray(inputs["w1"], dtype=np.float32))
    fc2wT = np.ascontiguousarray(np.asarray(inputs["fc2_w"], dtype=np.float32).T)
    fc3wT = np.ascontiguousarray(np.asarray(inputs["fc3_w"], dtype=np.float32).T)
    fc4wT = np.ascontiguousarray(np.asarray(inputs["fc4_w"], dtype=np.float32).T)
    fc2b = np.ascontiguousarray(np.asarray(inputs["fc2_b"], dtype=np.float32).reshape(MLPH, 1))
    fc3b = np.ascontiguousarray(np.asarray(inputs["fc3_b"], dtype=np.float32).reshape(MLPH, 1))
    fc4b = np.ascontiguousarray(np.asarray(inputs["fc4_b"], dtype=np.float32).reshape(1, 1))
    in_maps = []
    for i in range(NCORES):
        sl = slice(i * BLOC, (i + 1) * BLOC)
        in_maps.append(
            {
                "idx0": idx0[sl],
                "idx1": idx1[sl],
                "w1": w1,
                "fc2wT": fc2wT,
                "fc3wT": fc3wT,
                "fc4wT": fc4wT,
                "fc2b": fc2b,
                "fc3b": fc3b,
                "fc4b": fc4b,
            }
        )
    return in_maps


def run(inputs, trace=False, tmpdir=None):
    from concourse.bass_utils import run_bass_kernel_spmd

    if "nc" not in _COMPILED:
        _COMPILED["nc"] = _build_bass()
    nc = _COMPILED["nc"]
    in_maps = _prep_in_maps(inputs)
    res = run_bass_kernel_spmd(
        nc, in_maps, list(range(NCORES)), trace=trace, tmpdir=tmpdir
    )
    out = np.concatenate(
        [res.results[i]["out"].reshape(BLOC) for i in range(NCORES)]
    ).astype(np.float32)
    return out, res


def kernel(**inputs):
    out, _ = run(inputs, trace=False)
    return out
